# revision 1
# baseline (speedup 1.0000x reference)
"""CapsuleLayer (dynamic routing) Trainium2 kernel.

Problem: x [64,1152,8] f32, W [1152,64,8,16] f32 ->
  u_hat = einsum('bid,iodc->bioc', x, W)
  3 routing iterations (softmax over o=64, weighted i-sum, squash, agreement)
  returns v [64,64,16] f32.

Sharding: data-parallel over batch, 8 batch elements per core x 8 cores.

Per-core device strategy (raw bass, static program, manual semaphores):
  Phase 1: stream 72 fused tiles WX[g] = [W_tile | xbd_tile] fp16 [128,1152]
    (3-slot pipeline). PE: u_hat psum tiles (block-diag x) + s0 accumulation
    (uniform c0 folded into xdn/64). Evac psum->SBUF fp16 split ACT/DVE.
  Routing iters t=0,1 over 18 chunks of 4 groups; chunks are split between
    DVE (10) and GpSimd (8): owner does agr-mult + c-tree + logit update +
    s-mult; DVE does all softmax pieces; PE reduces partitions (selector)
    accumulating s in psum; ACT does exp. DVE pipeline is reordered
    (softmax of chunk k-1 after agr of chunk k) to hide ACT latency.
  Squash on ACT/DVE; v replicated to 128 partitions via 16 small DMAs.

Precision (validated vs f32 reference in numpy: rel err ~5e-4):
  fp16 inputs/u_hat/logits/exp/c, f32 psum accumulation and squash math.
"""

import numpy as np

NB = 8        # batch per core
NCORES = 8
G = 72        # i-groups of 16 in-capsules
CG = 4        # groups per routing chunk
CH = G // CG  # 18 chunks
O, C, D = 64, 16, 8
ISUB = 16     # in-caps per group
NWX = 4       # WX pipeline slots

POOL_CHUNKS = (1, 2, 3, 5, 6, 7, 9, 10, 11, 13, 14, 15)
PP = len(POOL_CHUNKS)          # 8 pool chunks per iter
DD = CH - PP                   # 10 dve chunks per iter


def _is_pool(k):
    return k in POOL_CHUNKS


def _cnt_p(k):
    return sum(1 for j in POOL_CHUNKS if j <= k)


def _cnt_d(k):
    return (k + 1) - _cnt_p(k)


_cache = {}
PARANOID = [False]  # True: emit same-engine drains for CoreSim race detector



def _build_program(paranoid=False):
    import concourse.bass as bass
    import concourse.mybir as mybir

    f16 = mybir.dt.float16
    f32 = mybir.dt.float32

    nc = bass.Bass('TRN2', target_bir_lowering=False, debug=False)

    # ---- DRAM I/O ----
    WX = nc.dram_tensor('WX', [G, 128, 1152], f16, kind='ExternalInput')
    XDN = nc.dram_tensor('XDN', [128, G * NB], f16, kind='ExternalInput')
    SEL = nc.dram_tensor('SEL', [128, NB], f16, kind='ExternalInput')
    VOUT = nc.dram_tensor('VOUT', [NB, 1024], f32, kind='ExternalOutput')

    # ---- SBUF ----
    u = nc.alloc_sbuf_tensor('u', [128, G * 1024], f16)          # 144KB/part
    wxs = [nc.alloc_sbuf_tensor('wx%d' % i, [128, 1152], f16)
           for i in range(NWX)]
    xdn = nc.alloc_sbuf_tensor('xdn', [128, G * NB], f16)
    sel = nc.alloc_sbuf_tensor('sel', [128, NB], f16)
    L = nc.alloc_sbuf_tensor('L', [128, G * O], f16)             # 9KB
    Ltmp = nc.alloc_sbuf_tensor('Ltmp', [128, CG * O], f16)
    cb = [nc.alloc_sbuf_tensor('cb%d' % i, [128, CG * O], f16)
          for i in range(2)]
    tmpa = nc.alloc_sbuf_tensor('tmpa', [128, CG * 1024], f16)   # 8KB
    tmps = [nc.alloc_sbuf_tensor('tmps%d' % i, [128, CG * 1024], f16)
            for i in range(2)]
    eb = [nc.alloc_sbuf_tensor('eb%d' % i, [128, CG * O], f16)
          for i in range(2)]
    Zb = nc.alloc_sbuf_tensor('Zb', [128, G], f32)
    zr = nc.alloc_sbuf_tensor('zr', [128, G], f32)
    vrep = nc.alloc_sbuf_tensor('vrep', [128, 1024], f16)
    v16 = nc.alloc_sbuf_tensor('v16', [NB, 1024], f16)
    s2 = nc.alloc_sbuf_tensor('s2', [NB, 1024], f32)             # also vf
    sq = nc.alloc_sbuf_tensor('sq', [NB, O], f32)
    rr = nc.alloc_sbuf_tensor('rr', [NB, O], f32)
    q1 = nc.alloc_sbuf_tensor('q1', [NB, O], f32)
    q2 = nc.alloc_sbuf_tensor('q2', [NB, O], f32)
    ff = nc.alloc_sbuf_tensor('ff', [NB, O], f32)
    vf = s2  # s2's last read (the sq reduce) precedes the vf write

    # ---- PSUM ----
    pg0 = nc.alloc_psum_tensor('pg0', [128, 1024], f32)
    pg1 = nc.alloc_psum_tensor('pg1', [128, 1024], f32)
    ps = nc.alloc_psum_tensor('ps', [NB, 1024], f32)
    pg = [pg0, pg1]

    AF = mybir.ActivationFunctionType
    AX = mybir.AxisListType

    sems = {}
    for name in ['d0', 'wxfree', 'pgsem', 'evsemA', 'evsemD',
                 'ssem', 'qa', 'qb', 'qc', 'sqdone', 'v16sem', 'vfsem',
                 'vrsem', 'Lsem_d', 'Lsem_p', 'xsem', 'ebfree', 'csem',
                 'smsem_d', 'smsem_p', 'tmpsfree', 'dout']:
        sems[name] = nc.alloc_semaphore(name)
    wxsems = [nc.alloc_semaphore('wxs%d' % i) for i in range(NWX)]
    S = type('S', (), sems)

    def ap3(t, base, dims):
        # strided view: dims = [(step, count), ...] on free axis
        a = t.ap()
        return bass.AP(a.tensor, base, [a.ap[0]] + [[s, n] for s, n in dims])

    def dr(eng):
        # same-engine RAW ordering is guaranteed by in-order engines with
        # per-op pipeline drain; explicit drains only appease the race
        # detector in CoreSim builds.
        if paranoid:
            eng.drain()

    def agr_block(eng, t, k, buf, ltbuf, lsem):
        """agreement mult + c-tree + logit update for chunk k, iter t."""
        ub = k * CG * 1024
        eng.tensor_mul(
            buf.ap(),
            ap3(u, ub, [(1024, CG), (1, 1024)]),
            ap3(vrep, 0, [(0, CG), (1, 1024)]))
        dr(eng)
        eng.tensor_add(
            ap3(buf, 0, [(1024, CG), (O, 8), (1, O)]),
            ap3(buf, 0, [(1024, CG), (O, 8), (1, O)]),
            ap3(buf, 512, [(1024, CG), (O, 8), (1, O)]))
        dr(eng)
        eng.tensor_add(
            ap3(buf, 0, [(1024, CG), (O, 4), (1, O)]),
            ap3(buf, 0, [(1024, CG), (O, 4), (1, O)]),
            ap3(buf, 256, [(1024, CG), (O, 4), (1, O)]))
        dr(eng)
        eng.tensor_add(
            ap3(buf, 0, [(1024, CG), (O, 2), (1, O)]),
            ap3(buf, 0, [(1024, CG), (O, 2), (1, O)]),
            ap3(buf, 128, [(1024, CG), (O, 2), (1, O)]))
        dr(eng)
        lsl = ap3(L, k * CG * O, [(O, CG), (1, O)])
        t3a = ap3(buf, 0, [(1024, CG), (1, O)])
        t3b = ap3(buf, O, [(1024, CG), (1, O)])
        if t == 0:
            eng.tensor_add(lsl, t3a, t3b).then_inc(lsem, 1)
        else:
            eng.tensor_add(ltbuf.ap(), t3a, t3b)
            dr(eng)
            eng.tensor_add(lsl, lsl, ltbuf.ap()).then_inc(lsem, 1)
        dr(eng)

    def smult(eng, t, k, smsem):
        """s-mult for chunk k: tmps[n%2] = u_chunk * c (bcast over c)."""
        n = t * CH + k
        if n >= 2:
            eng.wait_ge(S.tmpsfree, n - 1)
        eng.tensor_mul(
            tmps[n % 2].ap(),
            ap3(u, k * CG * 1024, [(1024, CG), (O, C), (1, O)]),
            ap3(cb[n % 2], 0, [(O, CG), (0, C), (1, O)])) \
            .then_inc(smsem, 1)
        dr(eng)

    with nc.allow_low_precision(reason='fp16 validated to 5e-4 vs f32 ref'), \
         nc.Block() as block:

        # ---------------- SYNC: all DMA ----------------
        @block.sync
        def _(eng):
            eng.dma_start(sel.ap(), SEL.ap()).then_inc(S.d0, 16)
            eng.dma_start(xdn.ap(), XDN.ap()).then_inc(S.d0, 16)
            for g in range(G):
                if g >= NWX:
                    eng.wait_ge(S.wxfree, g - NWX + 1)
                eng.dma_start(wxs[g % NWX].ap(), WX.ap()[g]) \
                   .then_inc(wxsems[g % NWX], 16)
            for t in range(2):
                eng.wait_ge(S.v16sem, t + 1)
                if t == 1:
                    eng.wait_ge(S.Lsem_d, CH)
                for isub in range(ISUB):
                    eng.dma_start(vrep.ap()[isub * NB:(isub + 1) * NB, :],
                                  v16.ap()).then_inc(S.vrsem, 16)
            eng.wait_ge(S.vfsem, 1)
            eng.dma_start(VOUT.ap(), vf.ap()).then_inc(S.dout, 16)

        # ---------------- PE ----------------
        @block.tensor
        def _(eng):
            for g in range(G):
                b = g % NWX
                eng.wait_ge(wxsems[b], 16 * (g // NWX + 1))
                if g >= 2:
                    gp = g - 2  # evac owner of pg slot being overwritten
                    if gp % 2 == 0:
                        eng.wait_ge(S.evsemA, gp // 2 + 1)
                    else:
                        eng.wait_ge(S.evsemD, (gp + 1) // 2)
                eng.matmul(pg[g % 2].ap()[:, 0:512],
                           lhsT=wxs[b].ap()[:, 1024:1152],
                           rhs=wxs[b].ap()[:, 0:512], start=True, stop=True)
                eng.matmul(pg[g % 2].ap()[:, 512:1024],
                           lhsT=wxs[b].ap()[:, 1024:1152],
                           rhs=wxs[b].ap()[:, 512:1024],
                           start=True, stop=True).then_inc(S.pgsem, 1)
                if g == 0:
                    eng.wait_ge(S.d0, 32)
                eng.matmul(ps.ap()[:, 0:512],
                           lhsT=xdn.ap()[:, g * NB:(g + 1) * NB],
                           rhs=wxs[b].ap()[:, 0:512],
                           start=(g == 0), stop=(g == G - 1))
                eng.matmul(ps.ap()[:, 512:1024],
                           lhsT=xdn.ap()[:, g * NB:(g + 1) * NB],
                           rhs=wxs[b].ap()[:, 512:1024],
                           start=(g == 0), stop=(g == G - 1)) \
                   .then_inc(S.wxfree, 1)
                if g == G - 1:
                    eng.maybe_drain_then_inc((S.ssem, 1))
            for t in range(2):
                eng.wait_ge(S.sqdone, t + 1)
                for k in range(CH):
                    n = t * CH + k
                    if _is_pool(k):
                        eng.wait_ge(S.smsem_p, PP * t + _cnt_p(k))
                    else:
                        eng.wait_ge(S.smsem_d, DD * t + _cnt_d(k))
                    for gs in range(CG):
                        for h in range(2):
                            mm = eng.matmul(
                                ps.ap()[:, h * 512:(h + 1) * 512],
                                lhsT=sel.ap(),
                                rhs=tmps[n % 2].ap()[:, gs * 1024 + h * 512:
                                                     gs * 1024 + (h + 1) * 512],
                                start=(k == 0 and gs == 0),
                                stop=(k == CH - 1 and gs == CG - 1))
                    mm.then_inc(S.tmpsfree, 1)
                    if k == CH - 1:
                        eng.maybe_drain_then_inc((S.ssem, 1))

        # ---------------- ACT (scalar) ----------------
        @block.scalar
        def _(eng):
            def squash_act(t):
                eng.wait_ge(S.ssem, t + 1)
                eng.activation(s2.ap(), ps.ap(), AF.Square).then_inc(S.qa, 1)
                eng.wait_ge(S.qb, t + 1)
                eng.activation(rr.ap(), sq.ap(), AF.Sqrt).then_inc(S.qc, 1)

            for g in range(0, G, 2):   # even g evac
                eng.wait_ge(S.pgsem, g + 1)
                eng.activation(ap3(u, g * 1024, [(1, 1024)]),
                               pg[g % 2].ap(), AF.Copy).then_inc(S.evsemA, 1)
            squash_act(0)
            for t in range(2):
                for k in range(CH):
                    n = t * CH + k
                    eng.wait_ge(S.Lsem_d, n + 1)
                    if n >= 2:
                        eng.wait_ge(S.ebfree, n - 1)
                    for gi in range(CG):
                        a = eng.activation(
                            ap3(eb[n % 2], gi * O, [(1, O)]),
                            ap3(L, k * CG * O + gi * O, [(1, O)]),
                            AF.Exp,
                            accum_out=ap3(Zb, k * CG + gi, [(1, 1)]))
                    a.then_inc(S.xsem, 1)
                squash_act(t + 1)

        # ---------------- DVE (vector) ----------------
        @block.vector
        def _(eng):
            def squash_dve(t):
                eng.wait_ge(S.qa, t + 1)
                eng.reduce_sum(sq.ap(),
                               ap3(s2, 0, [(1, O), (O, C)]),
                               axis=AX.X).then_inc(S.qb, 1)
                dr(eng)
                eng.wait_ge(S.qc, t + 1)
                eng.tensor_scalar_add(q1.ap(), sq.ap(), 1.0)
                eng.tensor_scalar_add(q2.ap(), rr.ap(), 1e-8)
                dr(eng)
                eng.tensor_mul(q1.ap(), q1.ap(), q2.ap())
                dr(eng)
                eng.reciprocal(q2.ap(), q1.ap())
                dr(eng)
                eng.tensor_mul(ff.ap(), sq.ap(), q2.ap())
                dr(eng)
                fb = ap3(ff, 0, [(0, C), (1, O)])
                if t < 2:
                    eng.tensor_mul(v16.ap(), ps.ap(), fb) \
                       .then_inc(S.v16sem, 1)
                    eng.maybe_drain_then_inc((S.sqdone, 1))
                else:
                    eng.tensor_mul(vf.ap(), ps.ap(), fb).then_inc(S.vfsem, 1)

            def softmax_smult(t, j):
                nj = t * CH + j
                eng.wait_ge(S.xsem, nj + 1)
                eng.reciprocal(ap3(zr, j * CG, [(1, CG)]),
                               ap3(Zb, j * CG, [(1, CG)]))
                dr(eng)
                j2 = j - 2  # cbuf slot WAR vs pool reader two chunks back
                if j2 >= 0 and _is_pool(j2):
                    eng.wait_ge(S.smsem_p, PP * t + _cnt_p(j2))
                eng.tensor_mul(cb[nj % 2].ap(),
                               ap3(eb[nj % 2], 0, [(O, CG), (1, O)]),
                               ap3(zr, j * CG, [(1, CG), (0, O)]))
                eng.sem_inc(S.ebfree, 1)
                eng.maybe_drain_then_inc((S.csem, 1))
                if not _is_pool(j):
                    smult(eng, t, j, S.smsem_d)

            for g in range(1, G, 2):   # odd g evac
                eng.wait_ge(S.pgsem, g + 1)
                eng.tensor_copy(ap3(u, g * 1024, [(1, 1024)]),
                                pg[g % 2].ap()).then_inc(S.evsemD, 1)
            squash_dve(0)
            for t in range(2):
                for k in range(CH):
                    if t == 0:
                        eng.wait_ge(S.evsemA, 2 * k + 2)
                        eng.wait_ge(S.evsemD, 2 * k + 2)
                    if k == 0:
                        eng.wait_ge(S.vrsem, 16 * ISUB * (t + 1))
                    agr_block(eng, t, k, tmpa, Ltmp, S.Lsem_d)
                    if k > 0:
                        softmax_smult(t, k - 1)
                softmax_smult(t, CH - 1)
                squash_dve(t + 1)

        # ---------------- GpSimd (pool): s-mults only ----------------
        @block.gpsimd
        def _(eng):
            for t in range(2):
                for k in POOL_CHUNKS:
                    n = t * CH + k
                    if t == 0:
                        eng.wait_ge(S.evsemA, 2 * k + 2)
                        eng.wait_ge(S.evsemD, 2 * k + 2)
                    eng.wait_ge(S.csem, n + 1)
                    smult(eng, t, k, S.smsem_p)

    return nc


def _preprocess(x, W):
    """Host-side repack (fp16 casts + layout) -> per-core input maps."""
    f16 = np.float16
    # W tiles: [g, (i_sub*8+d), (c*64+o)]
    Wt = np.ascontiguousarray(
        W.reshape(G, ISUB, O, D, C).transpose(0, 1, 3, 4, 2)
        .reshape(G, 128, 1024)).astype(f16)
    in_maps = []
    sel = np.zeros((128, NB), f16)
    sel[np.arange(128), np.arange(128) % NB] = 1.0
    for core in range(NCORES):
        xc = x[core * NB:(core + 1) * NB]            # [8, 1152, 8]
        xr = xc.reshape(NB, G, ISUB, D)              # (b, g, i_sub, d)
        xbd = np.zeros((G, 128, 128), f16)
        for isub in range(ISUB):
            xbd[:, isub * D:(isub + 1) * D, isub * NB:(isub + 1) * NB] = \
                xr[:, :, isub, :].transpose(1, 2, 0)  # (g, d, b)
        WXc = np.concatenate([Wt, xbd], axis=2)      # [72, 128, 1152]
        xdn = np.ascontiguousarray(
            (xr / 64.0).transpose(2, 3, 1, 0).reshape(128, G * NB)).astype(f16)
        in_maps.append({'WX': WXc, 'XDN': xdn, 'SEL': sel})
    return in_maps


def _postprocess(results):
    out = np.empty((NCORES * NB, O, C), np.float32)
    for core in range(NCORES):
        vo = results[core]['VOUT']                   # [8, 1024] = (c, o)
        out[core * NB:(core + 1) * NB] = \
            vo.reshape(NB, C, O).transpose(0, 2, 1)
    return out


def kernel(x, W):
    from concourse.bass_utils import run_bass_kernel_spmd
    x = np.asarray(x, np.float32)
    W = np.asarray(W, np.float32)
    if 'nc' not in _cache:
        # paranoid=True: same-engine drains are required on hardware too —
        # verified empirically (drain-stripped build returns garbage).
        _cache['nc'] = _build_program(paranoid=True)
    in_maps = _preprocess(x, W)
    res = run_bass_kernel_spmd(_cache['nc'], in_maps,
                               core_ids=list(range(NCORES)))
    return _postprocess(res.results)


def kernel_sim(x, W, core=0):
    """CoreSim single-core check: returns v for that core's 8 batch rows."""
    from concourse import bass_interp
    x = np.asarray(x, np.float32)
    W = np.asarray(W, np.float32)
    if 'nc_sim' not in _cache:
        _cache['nc_sim'] = _build_program(paranoid=True)
    in_maps = _preprocess(x, W)
    sim = bass_interp.CoreSim(_cache['nc_sim'])
    for name, arr in in_maps[core].items():
        sim.tensor(name)[:] = arr
    sim.simulate()
    vo = np.asarray(sim.tensor('VOUT'))
    return vo.reshape(NB, C, O).transpose(0, 2, 1)



# revision 7
# speedup vs baseline: 1.1384x; 1.1384x over previous
"""CapsuleLayer (dynamic routing) Trainium2 kernel.

Problem: x [64,1152,8] f32, W [1152,64,8,16] f32 ->
  u_hat = einsum('bid,iodc->bioc', x, W)
  3 routing iterations (softmax over o=64, weighted i-sum, squash, agreement)
  returns v [64,64,16] f32.

Sharding: data-parallel over batch, 8 batch elements per core x 8 cores.

Per-core device strategy (raw bass, static program, manual semaphores):
  Phase 1: stream 72 fused tiles WX[g] = [W_tile | xbd_tile] fp16 [128,1152]
    (4-slot pipeline). PE: u_hat psum tiles (block-diag x) + s0 accumulation
    (uniform c0 folded into xdn/64). Psum evac -> SBUF fp16 split 3-way
    ACT/DVE/Pool (g%3).
  Routing iters t=0,1 over 18 chunks of 4 groups. DVE: agreement mult +
    c-tree + logit update + reciprocal of Z. ACT: softmax exp (pass1 with
    f32 accum -> Z) and normalize (pass2 Copy with per-partition scale 1/Z)
    producing c directly. s-mult (tmps = u*c) split Pool (11 chunks) / DVE
    (7 chunks); PE reduces partitions (selector) accumulating s in psum.
  v-replicate: PE ones-matmul (rep8.T @ v16 -> psum) + ACT copy -> vrep,
    instead of per-isub DMAs.
  Squash on ACT/DVE; final vf -> VOUT DMA.

Precision (validated vs f32 reference in numpy: rel err ~5e-4):
  fp16 inputs/u_hat/logits/exp/c, f32 psum accumulation and squash math.
"""

import numpy as np

NB = 8        # batch per core
NCORES = 8
G = 72        # i-groups of 16 in-capsules
CG = 4        # groups per routing chunk
CH = G // CG  # 18 chunks
O, C, D = 64, 16, 8
ISUB = 16     # in-caps per group
NWX = 4       # WX pipeline slots
NCB = 4       # cb ring slots
NTS = 2       # tmps ring slots

# smult ownership: Pool takes k%3 != 0 except the last chunk (17) so the
# PE drain tail rides on a fast DVE smult.
POOL_CHUNKS = tuple(k for k in range(CH) if k % 3 != 0 and k != CH - 1)
PP = len(POOL_CHUNKS)          # 11 pool chunks per iter
DD = CH - PP                   # 7 dve chunks per iter

# GPSIMD cannot access PSUM (birverifier), so evac is ACT/DVE only.
EVAC = 'ADADADADA'             # evac owner rotation by g%9 (A:5, D:4)


def _is_pool(k):
    return k in POOL_CHUNKS


def _cnt_p(k):
    return sum(1 for j in POOL_CHUNKS if j <= k)


def _cnt_d(k):
    return (k + 1) - _cnt_p(k)


def _evac_owner(g):
    return EVAC[g % 9]


def _evac_cnt(owner, m):
    # number of groups <= m evac'd by `owner`
    if m < 0:
        return 0
    full, rem = divmod(m + 1, 9)
    return (full * EVAC.count(owner) +
            sum(1 for r in range(rem) if EVAC[r] == owner))


_cache = {}
PARANOID = [False]  # True: emit same-engine drains for CoreSim race detector


def _build_program(paranoid=False):
    import concourse.bass as bass
    import concourse.mybir as mybir

    f16 = mybir.dt.float16
    f32 = mybir.dt.float32

    nc = bass.Bass('TRN2', target_bir_lowering=False, debug=False)

    # ---- DRAM I/O ----
    WX = nc.dram_tensor('WX', [G, 128, 1152], f16, kind='ExternalInput')
    XDN = nc.dram_tensor('XDN', [128, G * NB], f16, kind='ExternalInput')
    SEL = nc.dram_tensor('SEL', [128, NB], f16, kind='ExternalInput')
    REP = nc.dram_tensor('REP', [NB, 128], f16, kind='ExternalInput')
    VOUT = nc.dram_tensor('VOUT', [NB, 1024], f32, kind='ExternalOutput')

    # ---- SBUF ----
    u = nc.alloc_sbuf_tensor('u', [128, G * 1024], f16)          # 144KB/part
    wxs = [nc.alloc_sbuf_tensor('wx%d' % i, [128, 1152], f16)
           for i in range(NWX)]
    xdn = nc.alloc_sbuf_tensor('xdn', [128, G * NB], f16)
    sel = nc.alloc_sbuf_tensor('sel', [128, NB], f16)
    rep8 = nc.alloc_sbuf_tensor('rep8', [NB, 128], f16)
    L = nc.alloc_sbuf_tensor('L', [128, G * O], f16)             # 9KB
    Ltmp = nc.alloc_sbuf_tensor('Ltmp', [128, CG * O], f16)
    eb = nc.alloc_sbuf_tensor('eb', [128, CG * O], f16)
    cb = [nc.alloc_sbuf_tensor('cb%d' % i, [128, CG * O], f16)
          for i in range(NCB)]
    tmpa = nc.alloc_sbuf_tensor('tmpa', [128, CG * 1024], f16)   # 8KB
    tmps = [nc.alloc_sbuf_tensor('tmps%d' % i, [128, CG * 1024], f16)
            for i in range(NTS)]
    Zb = nc.alloc_sbuf_tensor('Zb', [128, G], f32)
    zr = nc.alloc_sbuf_tensor('zr', [128, G], f32)
    vrep = nc.alloc_sbuf_tensor('vrep', [128, 1024], f16)
    v16 = nc.alloc_sbuf_tensor('v16', [NB, 1024], f16)
    s2 = nc.alloc_sbuf_tensor('s2', [NB, 1024], f32)             # also vf
    sq = nc.alloc_sbuf_tensor('sq', [NB, O], f32)
    rr = nc.alloc_sbuf_tensor('rr', [NB, O], f32)
    q1 = nc.alloc_sbuf_tensor('q1', [NB, O], f32)
    q2 = nc.alloc_sbuf_tensor('q2', [NB, O], f32)
    ff = nc.alloc_sbuf_tensor('ff', [NB, O], f32)
    vf = s2  # s2's last read (the sq reduce) precedes the vf write

    # ---- PSUM ----
    pg0 = nc.alloc_psum_tensor('pg0', [128, 1024], f32)
    pg1 = nc.alloc_psum_tensor('pg1', [128, 1024], f32)
    ps = nc.alloc_psum_tensor('ps', [NB, 1024], f32)
    pg = [pg0, pg1]

    AF = mybir.ActivationFunctionType
    AX = mybir.AxisListType

    sems = {}
    for name in ['d0', 'wxfree', 'pgsem', 'evA', 'evD', 'evP',
                 'ssem', 'qa', 'qb', 'qc', 'v16sem', 'vfsem',
                 'repsem', 'vrsem', 'Lsem', 'xsem', 'zrsem', 'csem',
                 'smsem_d', 'smsem_p', 'tmpsfree', 'dout']:
        sems[name] = nc.alloc_semaphore(name)
    wxsems = [nc.alloc_semaphore('wxs%d' % i) for i in range(NWX)]
    S = type('S', (), sems)
    EVS = {'A': S.evA, 'D': S.evD, 'P': S.evP}

    def ap3(t, base, dims):
        # strided view: dims = [(step, count), ...] on free axis
        a = t.ap()
        return bass.AP(a.tensor, base, [a.ap[0]] + [[s, n] for s, n in dims])

    def dr(eng):
        # same-engine RAW ordering within the ack window needs drains on HW;
        # also appeases the CoreSim race detector.
        if paranoid:
            eng.drain()

    def wait_evac(eng, m):
        # wait until groups 0..m are evacuated to SBUF u
        for o in 'AD':
            c = _evac_cnt(o, m)
            if c > 0:
                eng.wait_ge(EVS[o], c)

    def agr_block(eng, t, k):
        """agreement mult + c-tree + logit update for chunk k, iter t."""
        ub = k * CG * 1024
        eng.tensor_mul(
            tmpa.ap(),
            ap3(u, ub, [(1024, CG), (1, 1024)]),
            ap3(vrep, 0, [(0, CG), (1, 1024)]))
        dr(eng)
        eng.tensor_add(
            ap3(tmpa, 0, [(1024, CG), (O, 8), (1, O)]),
            ap3(tmpa, 0, [(1024, CG), (O, 8), (1, O)]),
            ap3(tmpa, 512, [(1024, CG), (O, 8), (1, O)]))
        dr(eng)
        eng.tensor_add(
            ap3(tmpa, 0, [(1024, CG), (O, 4), (1, O)]),
            ap3(tmpa, 0, [(1024, CG), (O, 4), (1, O)]),
            ap3(tmpa, 256, [(1024, CG), (O, 4), (1, O)]))
        dr(eng)
        eng.tensor_add(
            ap3(tmpa, 0, [(1024, CG), (O, 2), (1, O)]),
            ap3(tmpa, 0, [(1024, CG), (O, 2), (1, O)]),
            ap3(tmpa, 128, [(1024, CG), (O, 2), (1, O)]))
        dr(eng)
        lsl = ap3(L, k * CG * O, [(O, CG), (1, O)])
        t3a = ap3(tmpa, 0, [(1024, CG), (1, O)])
        t3b = ap3(tmpa, O, [(1024, CG), (1, O)])
        if t == 0:
            eng.tensor_add(lsl, t3a, t3b).then_inc(S.Lsem, 1)
        else:
            eng.tensor_add(Ltmp.ap(), t3a, t3b)
            dr(eng)
            eng.tensor_add(lsl, lsl, Ltmp.ap()).then_inc(S.Lsem, 1)
        dr(eng)

    def smult(eng, t, k, smsem):
        """s-mult for chunk k: tmps[ring] = u_chunk * c (bcast over c)."""
        n = t * CH + k
        if n >= NTS:
            eng.wait_ge(S.tmpsfree, n - NTS + 1)
        eng.wait_ge(S.csem, n + 1)
        eng.tensor_mul(
            tmps[n % NTS].ap(),
            ap3(u, k * CG * 1024, [(1024, CG), (O, C), (1, O)]),
            ap3(cb[n % NCB], 0, [(O, CG), (0, C), (1, O)])) \
            .then_inc(smsem, 1)
        dr(eng)

    with nc.allow_low_precision(reason='fp16 validated to 5e-4 vs f32 ref'), \
         nc.Block() as block:

        # ---------------- SYNC: all DMA ----------------
        @block.sync
        def _(eng):
            eng.dma_start(sel.ap(), SEL.ap()).then_inc(S.d0, 16)
            eng.dma_start(rep8.ap(), REP.ap()).then_inc(S.d0, 16)
            eng.dma_start(xdn.ap(), XDN.ap()).then_inc(S.d0, 16)
            for g in range(G):
                if g >= NWX:
                    eng.wait_ge(S.wxfree, g - NWX + 1)
                eng.dma_start(wxs[g % NWX].ap(), WX.ap()[g]) \
                   .then_inc(wxsems[g % NWX], 16)
            eng.wait_ge(S.vfsem, 1)
            eng.dma_start(VOUT.ap(), vf.ap()).then_inc(S.dout, 16)

        # ---------------- PE ----------------
        @block.tensor
        def _(eng):
            for g in range(G):
                b = g % NWX
                eng.wait_ge(wxsems[b], 16 * (g // NWX + 1))
                if g >= 2:
                    gp = g - 2  # evac owner of pg slot being overwritten
                    eng.wait_ge(EVS[_evac_owner(gp)],
                                _evac_cnt(_evac_owner(gp), gp))
                eng.matmul(pg[g % 2].ap()[:, 0:512],
                           lhsT=wxs[b].ap()[:, 1024:1152],
                           rhs=wxs[b].ap()[:, 0:512], start=True, stop=True)
                eng.matmul(pg[g % 2].ap()[:, 512:1024],
                           lhsT=wxs[b].ap()[:, 1024:1152],
                           rhs=wxs[b].ap()[:, 512:1024],
                           start=True, stop=True).then_inc(S.pgsem, 1)
                if g == 0:
                    eng.wait_ge(S.d0, 48)
                eng.matmul(ps.ap()[:, 0:512],
                           lhsT=xdn.ap()[:, g * NB:(g + 1) * NB],
                           rhs=wxs[b].ap()[:, 0:512],
                           start=(g == 0), stop=(g == G - 1))
                eng.matmul(ps.ap()[:, 512:1024],
                           lhsT=xdn.ap()[:, g * NB:(g + 1) * NB],
                           rhs=wxs[b].ap()[:, 512:1024],
                           start=(g == 0), stop=(g == G - 1)) \
                   .then_inc(S.wxfree, 1)
                if g == G - 1:
                    eng.maybe_drain_then_inc((S.ssem, 1))
            for t in range(2):
                # replicate v16 across partitions: pg0 = rep8.T @ v16
                eng.wait_ge(S.v16sem, t + 1)
                if t >= 1:
                    eng.wait_ge(S.vrsem, 16 * t)  # ACT done reading pg0
                for h in range(2):
                    eng.matmul(pg0.ap()[:, h * 512:(h + 1) * 512],
                               lhsT=rep8.ap(),
                               rhs=v16.ap()[:, h * 512:(h + 1) * 512],
                               start=True, stop=True)
                eng.maybe_drain_then_inc((S.repsem, 1))
                for k in range(CH):
                    n = t * CH + k
                    if _is_pool(k):
                        eng.wait_ge(S.smsem_p, PP * t + _cnt_p(k))
                    else:
                        eng.wait_ge(S.smsem_d, DD * t + _cnt_d(k))
                    for gs in range(CG):
                        for h in range(2):
                            mm = eng.matmul(
                                ps.ap()[:, h * 512:(h + 1) * 512],
                                lhsT=sel.ap(),
                                rhs=tmps[n % NTS].ap()[:, gs * 1024 + h * 512:
                                                       gs * 1024 + (h + 1) * 512],
                                start=(k == 0 and gs == 0),
                                stop=(k == CH - 1 and gs == CG - 1))
                    mm.then_inc(S.tmpsfree, 1)
                    if k == CH - 1:
                        eng.maybe_drain_then_inc((S.ssem, 1))

        # ---------------- ACT (scalar) ----------------
        @block.scalar
        def _(eng):
            def squash_act(t):
                eng.wait_ge(S.ssem, t + 1)
                eng.activation(s2.ap(), ps.ap(), AF.Square).then_inc(S.qa, 1)
                eng.wait_ge(S.qb, t + 1)
                eng.activation(rr.ap(), sq.ap(), AF.Sqrt).then_inc(S.qc, 1)

            for g in range(G):
                if _evac_owner(g) != 'A':
                    continue
                eng.wait_ge(S.pgsem, g + 1)
                eng.activation(ap3(u, g * 1024, [(1, 1024)]),
                               pg[g % 2].ap(), AF.Copy).then_inc(S.evA, 1)
            squash_act(0)
            for t in range(2):
                # v-replicate evac: vrep = f16(pg0)
                eng.wait_ge(S.repsem, t + 1)
                eng.activation(vrep.ap(), pg0.ap(), AF.Copy) \
                   .then_inc(S.vrsem, 16)
                for k in range(CH):
                    n = t * CH + k
                    eng.wait_ge(S.Lsem, n + 1)
                    # pass 1: eb = exp(L chunk), Zb column accum per gi
                    for gi in range(CG):
                        a = eng.activation(
                            ap3(eb, gi * O, [(1, O)]),
                            ap3(L, k * CG * O + gi * O, [(1, O)]),
                            AF.Exp,
                            accum_out=ap3(Zb, k * CG + gi, [(1, 1)]))
                    a.then_inc(S.xsem, 1)
                    dr(eng)
                    # pass 2: cb = eb * (1/Z)  (per-partition scale)
                    eng.wait_ge(S.zrsem, n + 1)
                    if n >= NCB:
                        # cb slot WAR vs its previous reader (owner of n-NCB)
                        kp = (n - NCB) % CH
                        tp = (n - NCB) // CH
                        if _is_pool(kp):
                            eng.wait_ge(S.smsem_p, PP * tp + _cnt_p(kp))
                        else:
                            eng.wait_ge(S.smsem_d, DD * tp + _cnt_d(kp))
                    for gi in range(CG):
                        a = eng.activation(
                            ap3(cb[n % NCB], gi * O, [(1, O)]),
                            ap3(eb, gi * O, [(1, O)]),
                            AF.Copy,
                            scale=ap3(zr, k * CG + gi, [(1, 1)]))
                    a.then_inc(S.csem, 1)
                    dr(eng)
                squash_act(t + 1)

        # ---------------- DVE (vector) ----------------
        @block.vector
        def _(eng):
            def squash_dve(t):
                eng.wait_ge(S.qa, t + 1)
                eng.reduce_sum(sq.ap(),
                               ap3(s2, 0, [(1, O), (O, C)]),
                               axis=AX.X).then_inc(S.qb, 1)
                dr(eng)
                eng.wait_ge(S.qc, t + 1)
                eng.tensor_scalar_add(q1.ap(), sq.ap(), 1.0)
                eng.tensor_scalar_add(q2.ap(), rr.ap(), 1e-8)
                dr(eng)
                eng.tensor_mul(q1.ap(), q1.ap(), q2.ap())
                dr(eng)
                eng.reciprocal(q2.ap(), q1.ap())
                dr(eng)
                eng.tensor_mul(ff.ap(), sq.ap(), q2.ap())
                dr(eng)
                fb = ap3(ff, 0, [(0, C), (1, O)])
                if t < 2:
                    eng.tensor_mul(v16.ap(), ps.ap(), fb) \
                       .then_inc(S.v16sem, 1)
                else:
                    eng.tensor_mul(vf.ap(), ps.ap(), fb).then_inc(S.vfsem, 1)
                dr(eng)

            def recip(t, k):
                eng.wait_ge(S.xsem, t * CH + k + 1)
                eng.reciprocal(ap3(zr, k * CG, [(1, CG)]),
                               ap3(Zb, k * CG, [(1, CG)]))
                eng.maybe_drain_then_inc((S.zrsem, 1))

            for g in range(G):
                if _evac_owner(g) != 'D':
                    continue
                eng.wait_ge(S.pgsem, g + 1)
                eng.tensor_copy(ap3(u, g * 1024, [(1, 1024)]),
                                pg[g % 2].ap()).then_inc(S.evD, 1)
            squash_dve(0)
            for t in range(2):
                for k in range(CH):
                    if k == 0:
                        eng.wait_ge(S.vrsem, 16 * (t + 1))
                    if t == 0:
                        wait_evac(eng, 4 * k + 3)
                    if t >= 1:
                        # L chunk WAR vs ACT pass1 of previous iter
                        eng.wait_ge(S.xsem, (t - 1) * CH + k + 1)
                    agr_block(eng, t, k)
                    if k >= 1:
                        recip(t, k - 1)
                    k2 = k - 2
                    if k2 >= 0 and not _is_pool(k2):
                        smult(eng, t, k2, S.smsem_d)
                recip(t, CH - 1)
                for k2 in (CH - 2, CH - 1):
                    if not _is_pool(k2):
                        smult(eng, t, k2, S.smsem_d)
                squash_dve(t + 1)

        # ---------------- GpSimd (pool) ----------------
        @block.gpsimd
        def _(eng):
            for t in range(2):
                for k in POOL_CHUNKS:
                    if t == 0:
                        wait_evac(eng, 4 * k + 3)
                    smult(eng, t, k, S.smsem_p)

    return nc


def _preprocess(x, W):
    """Host-side repack (fp16 casts + layout) -> per-core input maps."""
    f16 = np.float16
    # W tiles: [g, (i_sub*8+d), (c*64+o)]
    Wt = np.ascontiguousarray(
        W.reshape(G, ISUB, O, D, C).transpose(0, 1, 3, 4, 2)
        .reshape(G, 128, 1024)).astype(f16)
    in_maps = []
    sel = np.zeros((128, NB), f16)
    sel[np.arange(128), np.arange(128) % NB] = 1.0
    rep = np.zeros((NB, 128), f16)
    rep[np.arange(128) % NB, np.arange(128)] = 1.0
    for core in range(NCORES):
        xc = x[core * NB:(core + 1) * NB]            # [8, 1152, 8]
        xr = xc.reshape(NB, G, ISUB, D)              # (b, g, i_sub, d)
        xbd = np.zeros((G, 128, 128), f16)
        for isub in range(ISUB):
            xbd[:, isub * D:(isub + 1) * D, isub * NB:(isub + 1) * NB] = \
                xr[:, :, isub, :].transpose(1, 2, 0)  # (g, d, b)
        WXc = np.concatenate([Wt, xbd], axis=2)      # [72, 128, 1152]
        xdn = np.ascontiguousarray(
            (xr / 64.0).transpose(2, 3, 1, 0).reshape(128, G * NB)).astype(f16)
        in_maps.append({'WX': WXc, 'XDN': xdn, 'SEL': sel, 'REP': rep})
    return in_maps


def _postprocess(results):
    out = np.empty((NCORES * NB, O, C), np.float32)
    for core in range(NCORES):
        vo = results[core]['VOUT']                   # [8, 1024] = (c, o)
        out[core * NB:(core + 1) * NB] = \
            vo.reshape(NB, C, O).transpose(0, 2, 1)
    return out


def kernel(x, W):
    from concourse.bass_utils import run_bass_kernel_spmd
    x = np.asarray(x, np.float32)
    W = np.asarray(W, np.float32)
    if 'nc' not in _cache:
        # paranoid=True: same-engine drains are required on hardware too —
        # verified empirically (drain-stripped build returns garbage).
        _cache['nc'] = _build_program(paranoid=True)
    in_maps = _preprocess(x, W)
    res = run_bass_kernel_spmd(_cache['nc'], in_maps,
                               core_ids=list(range(NCORES)))
    return _postprocess(res.results)


def kernel_sim(x, W, core=0):
    """CoreSim single-core check: returns v for that core's 8 batch rows."""
    from concourse import bass_interp
    x = np.asarray(x, np.float32)
    W = np.asarray(W, np.float32)
    if 'nc_sim' not in _cache:
        _cache['nc_sim'] = _build_program(paranoid=True)
    in_maps = _preprocess(x, W)
    sim = bass_interp.CoreSim(_cache['nc_sim'])
    for name, arr in in_maps[core].items():
        sim.tensor(name)[:] = arr
    sim.simulate()
    vo = np.asarray(sim.tensor('VOUT'))
    return vo.reshape(NB, C, O).transpose(0, 2, 1)
